# revision 24
# baseline (speedup 1.0000x reference)
"""AdaptiveEdgeWeightGNN (GCNConv with edge weights) on 8 Trainium2 NeuronCores.

Strategy: destination-sharded edge-parallel with an ELLPACK slot grid.
Nodes are ranked by merged in-degree globally; rank r -> core r%8, window
(r//8)//128, partition (r//8)%128 — so all cores share one tight window
profile (one SPMD program). Host does pure index/layout work (self-loops,
parallel-edge merge, slot assignment, int16 pair indices into a [25088, 128]
paired view of the bf16 y-table). Device: degree reduce -> dis = deg^-1/2 ->
y = dis*x (own slice, bf16) -> AllGather -> staged dma_gather (4 SWDGE
queues) -> DVE pair-select scale -> TensorE identity-stationary PSUM
accumulation -> transpose -> @W -> dis-scale + bias.
"""
import os
import ml_dtypes
import numpy as np

import concourse.bacc as bacc
import concourse.bass as bass
import concourse.tile as tile
from concourse import mybir
from concourse.bass_utils import run_bass_kernel_spmd
from concourse.masks import make_identity

N_NODES = 50000
N_EDGES = 800000
D = 64
N_CORES = 8
NPC = 6250            # real nodes per core
PADN = 6272           # padded nodes per core (49 windows x 128)
N_WIN = PADN // 128   # 49
P = 128
CC = 16               # gather-call width in slot-columns

F32 = mybir.dt.float32
BF16 = mybir.dt.bfloat16
I16 = mybir.dt.int16


def _preprocess(edge_index: np.ndarray, edge_weight: np.ndarray):
    """Pure index/layout preprocessing. Returns per-core grids + permutation."""
    row = np.asarray(edge_index[0], dtype=np.int64)
    col = np.asarray(edge_index[1], dtype=np.int64)
    ew = np.asarray(edge_weight, dtype=np.float64)

    # self-loops (weight 1.0) — matches gcn_norm's add_self_loops
    loop = np.arange(N_NODES, dtype=np.int64)
    row = np.concatenate([row, loop])
    col = np.concatenate([col, loop])
    ew = np.concatenate([ew, np.ones(N_NODES)])

    # merge parallel edges by (dst, src)
    key = col * N_NODES + row
    order0 = np.argsort(key, kind="stable")
    ks = key[order0]
    uniq_mask = np.empty(ks.shape, dtype=bool)
    uniq_mask[0] = True
    uniq_mask[1:] = ks[1:] != ks[:-1]
    seg_id = np.cumsum(uniq_mask) - 1
    ew_m = np.bincount(seg_id, weights=ew[order0])
    ku = ks[uniq_mask]
    dst_m = ku // N_NODES
    src_m = ku % N_NODES

    # merged in-degree (slot count) per node; global degree ranking
    cnt = np.bincount(dst_m, minlength=N_NODES)
    grank_order = np.argsort(-cnt, kind="stable")   # rank -> node
    grank = np.empty(N_NODES, dtype=np.int64)       # node -> rank
    grank[grank_order] = np.arange(N_NODES)

    owner = grank % N_CORES                         # node -> core
    lrank = grank // N_CORES                        # node -> local rank (< 6250)
    perm = owner * PADN + lrank                     # node -> y-table row

    # common per-window K: global sorted counts, max of each 1024-stripe
    csort = cnt[grank_order]                        # descending
    K = np.zeros(N_WIN, dtype=np.int64)
    for w in range(N_WIN):
        s = w * 128 * N_CORES
        K[w] = csort[s] if s < N_NODES else 1
    K = np.maximum(K, 1)
    off = np.zeros(N_WIN + 1, dtype=np.int64)
    off[1:] = np.cumsum(K)
    cols_raw = int(off[-1])
    COLS = ((cols_raw + CC - 1) // CC) * CC

    # slot fill
    gidx = np.zeros((N_CORES, P, COLS), dtype=np.int16)
    slo = np.zeros((N_CORES, P, COLS), dtype=np.float32)
    shi = np.zeros((N_CORES, P, COLS), dtype=np.float32)

    own_m = owner[dst_m]
    lr_m = lrank[dst_m]
    wn = lr_m // 128
    pp = lr_m - wn * 128
    dst_seg_start = np.searchsorted(dst_m, dst_m)   # dst_m sorted
    j = np.arange(dst_m.size) - dst_seg_start
    colpos = off[wn] + j
    assert (j < K[wn]).all()

    prow = perm[src_m]
    pidx = (prow >> 1).astype(np.int16)
    par = (prow & 1).astype(bool)
    gidx[own_m, pp, colpos] = pidx
    slo[own_m, pp, colpos] = np.where(~par, ew_m, 0.0).astype(np.float32)
    shi[own_m, pp, colpos] = np.where(par, ew_m, 0.0).astype(np.float32)
    s2 = np.stack([slo, shi], axis=-1)          # [8, P, COLS, 2]

    # call plan: CC-wide calls, but split the last 16 columns into 8
    # two-column calls so the end-of-stream drain trickle is short
    call_plan = []
    c = 0
    while c < COLS:
        rem = COLS - c
        cc = CC if rem > 16 else 2
        call_plan.append((c, cc))
        c += cc
    ncalls = len(call_plan)
    # wrapped int16 index layout per gather call, concatenated
    parts = []
    for (c0, cc) in call_plan:
        blk = gidx[:, :, c0:c0 + cc]                       # [8, 128, cc]
        flat = blk.transpose(0, 2, 1).reshape(N_CORES, cc * 128)
        w16 = flat.reshape(N_CORES, cc * 8, 16).transpose(0, 2, 1)  # [8,16,cc*8]
        parts.append(np.tile(w16, (1, 8, 1)))
    gidx_w = np.concatenate(parts, axis=2)

    # rank -> node map per core for upload/unshard
    node_at_rank = np.full((N_CORES, PADN), -1, dtype=np.int64)
    for c in range(N_CORES):
        node_at_rank[c, :NPC] = grank_order[c::N_CORES]

    return dict(
        COLS=COLS, K=K, off=off, ncalls=ncalls, call_plan=call_plan,
        gidx_w=gidx_w, s2=s2,
        node_at_rank=node_at_rank,
    )


def _build_nc(COLS: int, off: np.ndarray, call_plan):
    nc = bacc.Bacc("TRN2", target_bir_lowering=False, debug=False,
                   num_devices=N_CORES, num_swdge_queues=4)
    x_in = nc.dram_tensor("x", [PADN, D], F32, kind="ExternalInput")
    gi_in = nc.dram_tensor("gidx", [P, COLS * 8], I16, kind="ExternalInput")
    s2_in = nc.dram_tensor("s2", [P, COLS, 2, 1], BF16, kind="ExternalInput")
    w_in = nc.dram_tensor("W", [D, D], F32, kind="ExternalInput")
    b_in = nc.dram_tensor("bias", [1, D], F32, kind="ExternalInput")
    out_t = nc.dram_tensor("out", [PADN, D], F32, kind="ExternalOutput")

    with tile.TileContext(nc) as tc:
        with tc.tile_pool(name="const", bufs=1) as cp, \
             tc.tile_pool(name="work", bufs=1) as wp, \
             tc.tile_pool(name="work2", bufs=2) as wp2, \
             tc.tile_pool(name="gq", bufs=8) as gq, \
             tc.tile_pool(name="mq", bufs=8) as mq, \
             tc.tile_pool(name="pa", bufs=1, space="PSUM") as pa, \
             tc.tile_pool(name="ps", bufs=1, space="PSUM") as ps, \
             tc.tile_pool(name="dram", bufs=1, space="DRAM") as dp:

            # ---- constants / small uploads ----
            ident_f = cp.tile([P, P], F32, tag="idf")
            make_identity(nc, ident_f[:])
            ident_b = cp.tile([P, P], BF16, tag="idb")
            nc.vector.tensor_copy(ident_b[:], ident_f[:])
            w_sb = cp.tile([D, D], F32, tag="w")
            nc.sync.dma_start(w_sb[:], w_in[:])
            b_row = cp.tile([1, D], F32, tag="brow")
            nc.sync.dma_start(b_row[:], b_in[:])
            ones1 = cp.tile([1, P], F32, tag="ones")
            nc.vector.memset(ones1[:], 1.0)
            bias_ps = ps.tile([P, D], F32, tag="small")
            nc.tensor.matmul(out=bias_ps[:], lhsT=ones1[:], rhs=b_row[:],
                             start=True, stop=True)
            bias_bc = cp.tile([P, 1, D], F32, tag="biasbc")
            nc.vector.tensor_copy(bias_bc[:, 0, :], bias_ps[:])

            s2_t = cp.tile([P, COLS, 2, 1], BF16, tag="s2")
            nc.sync.dma_start(s2_t[:], s2_in[:])
            gi_t = cp.tile([P, COLS * 8], I16, tag="gi")
            nc.sync.dma_start(gi_t[:], gi_in[:])

            # ---- degree / dis ----
            ew_tot = wp.tile([P, COLS, 1], F32, tag="ewt")
            nc.vector.tensor_tensor(out=ew_tot[:], in0=s2_t[:, :, 0, :],
                                    in1=s2_t[:, :, 1, :],
                                    op=mybir.AluOpType.add)
            deg = wp.tile([P, N_WIN, 1], F32, tag="deg")
            for w in range(N_WIN):
                nc.vector.tensor_reduce(
                    out=deg[:, w, :], in_=ew_tot[:, int(off[w]):int(off[w + 1]), :],
                    axis=mybir.AxisListType.XY, op=mybir.AluOpType.add)
            mask = wp.tile([P, N_WIN, 1], F32, tag="mask")
            nc.vector.tensor_scalar(out=mask[:], in0=deg[:], scalar1=0.0,
                                    scalar2=None, op0=mybir.AluOpType.is_gt)
            degp = wp.tile([P, N_WIN, 1], F32, tag="degp")
            nc.vector.tensor_scalar(out=degp[:], in0=deg[:], scalar1=-1.0,
                                    scalar2=None, op0=mybir.AluOpType.add)
            nc.vector.tensor_tensor(out=degp[:], in0=degp[:], in1=mask[:],
                                    op=mybir.AluOpType.mult)
            nc.vector.tensor_scalar(out=degp[:], in0=degp[:], scalar1=1.0,
                                    scalar2=None, op0=mybir.AluOpType.add)
            rec = wp.tile([P, N_WIN, 1], F32, tag="rec")
            nc.vector.reciprocal(rec[:], degp[:])
            dis = wp.tile([P, N_WIN, 1], F32, tag="dis")
            nc.scalar.sqrt(dis[:], rec[:])
            nc.vector.tensor_tensor(out=dis[:], in0=dis[:], in1=mask[:],
                                    op=mybir.AluOpType.mult)

            # ---- y slice build (bf16) + AllGather ----
            x_sb = wp.tile([P, N_WIN, D], F32, tag="xsb")
            nc.sync.dma_start(
                x_sb[:], x_in[:].rearrange("(w p) f -> p w f", p=P))
            y_sb = wp.tile([P, N_WIN, D], BF16, tag="ysb")
            nc.vector.tensor_tensor(out=y_sb[:], in0=x_sb[:],
                                    in1=dis[:].to_broadcast([P, N_WIN, D]),
                                    op=mybir.AluOpType.mult)
            ag_in = dp.tile([PADN, D], BF16, tag="agin")
            nc.gpsimd.dma_start(
                ag_in[:].rearrange("(w p) f -> p w f", p=P), y_sb[:])
            y_full = dp.tile([N_CORES * PADN, D], BF16, tag="yfull")
            nc.gpsimd.collective_compute(
                "AllGather", mybir.AluOpType.bypass,
                replica_groups=[list(range(N_CORES))],
                ins=[ag_in.opt()], outs=[y_full.opt()])
            y_pair = y_full[:].rearrange("(a two) f -> a (two f)", two=2)

            agg = pa.tile([P, N_WIN * D], F32, tag="agg")
            out_raw = wp.tile([P, N_WIN, D], F32, tag="outraw")
            first_col = {int(off[w]): w for w in range(N_WIN)}
            last_col = {int(off[w + 1]) - 1: w for w in range(N_WIN)}
            WPB = 8  # windows per PSUM bank
            n_banks = (N_WIN + WPB - 1) // WPB

            def bank_tail(b):
                """Evict retired bank b, transpose, @W, dis-scale, bias, DMA."""
                w0 = b * WPB
                bw = min(WPB, N_WIN - w0)
                tmp = wp2.tile([P, WPB * D], F32, tag="tmpagg")
                nc.vector.tensor_copy(tmp[:, :bw * D],
                                      agg[:, w0 * D:(w0 + bw) * D])
                for s0 in range(0, bw, 4):
                    sw = min(4, bw - s0)
                    pt = ps.tile([D, 4 * P], F32, tag="small")
                    for i in range(sw):
                        nc.tensor.transpose(
                            out=pt[:, i * P:(i + 1) * P],
                            in_=tmp[:, (s0 + i) * D:(s0 + i + 1) * D],
                            identity=ident_f[:])
                    at = wp2.tile([D, 4 * P], F32, tag="aggT")
                    nc.vector.tensor_copy(at[:, :sw * P], pt[:, :sw * P])
                    for i in range(sw):
                        w = w0 + s0 + i
                        nc.tensor.matmul(out=agg[:, w * D:(w + 1) * D],
                                         lhsT=at[:, i * P:(i + 1) * P],
                                         rhs=w_sb[:], start=True, stop=True)
                nc.vector.tensor_copy(
                    out_raw[:, w0:w0 + bw, :].rearrange("p w f -> p (w f)"),
                    agg[:, w0 * D:(w0 + bw) * D])
                nc.vector.tensor_tensor(
                    out=out_raw[:, w0:w0 + bw, :],
                    in0=out_raw[:, w0:w0 + bw, :],
                    in1=dis[:, w0:w0 + bw, :].to_broadcast([P, bw, D]),
                    op=mybir.AluOpType.mult)
                nc.vector.tensor_tensor(
                    out=out_raw[:, w0:w0 + bw, :],
                    in0=out_raw[:, w0:w0 + bw, :],
                    in1=bias_bc[:].to_broadcast([P, bw, D]),
                    op=mybir.AluOpType.add)
                nc.sync.dma_start(
                    out_t[:].rearrange("(w p) f -> p w f", p=P)[:, w0:w0 + bw, :],
                    out_raw[:, w0:w0 + bw, :])

            next_bank = 0
            gi_off = 0
            for k, (c0, cc) in enumerate(call_plan):
                g = gq.tile([P, CC, 2 * D], BF16, tag="g")
                nc.gpsimd.dma_gather(
                    out_ap=g[:, :cc, :], in_ap=y_pair,
                    idxs_ap=gi_t[:, gi_off:gi_off + cc * 8],
                    num_idxs=cc * P, num_idxs_reg=cc * P,
                    elem_size=2 * D, single_packet=False, queue_num=k % 4)
                gi_off += cc * 8
                m = mq.tile([P, CC, 2, D], BF16, tag="m")
                nc.vector.tensor_tensor(
                    out=m[:, :cc, :, :],
                    in0=g[:, :cc, :].rearrange("p c (two f) -> p c two f", two=2),
                    in1=s2_t[:, c0:c0 + cc, :, :].to_broadcast([P, cc, 2, D]),
                    op=mybir.AluOpType.mult)
                for cl in range(cc):
                    col = c0 + cl
                    if col >= int(off[-1]):
                        break
                    w = int(np.searchsorted(off, col, side="right")) - 1
                    nc.tensor.matmul(out=agg[:, w * D:(w + 1) * D],
                                     lhsT=ident_b[:], rhs=m[:, cl, 0, :],
                                     start=(col in first_col), stop=False)
                    nc.tensor.matmul(out=agg[:, w * D:(w + 1) * D],
                                     lhsT=ident_b[:], rhs=m[:, cl, 1, :],
                                     start=False, stop=(col in last_col))
                # emit tails for banks fully retired by this call
                while next_bank < n_banks:
                    wlast = min((next_bank + 1) * WPB, N_WIN) - 1
                    if int(off[wlast + 1]) <= min(c0 + cc, int(off[-1])):
                        bank_tail(next_bank)
                        next_bank += 1
                    else:
                        break
            # keep the Q7 SWDGE service loop hot while the last gather
            # drains complete (idle-loop doorbell pacing workaround)
            hot = wp2.tile([P, 32], F32, tag="hot")
            for _ in range(10):
                nc.gpsimd.dma_start(hot[:], ident_f[:, 0:32])
            while next_bank < n_banks:
                bank_tail(next_bank)
                next_bank += 1

    nc.compile()
    return nc


_CACHE: dict = {}


def kernel(x, W, bias, edge_weight, edge_index) -> np.ndarray:
    x = np.asarray(x, dtype=np.float32)
    W = np.asarray(W, dtype=np.float32)
    bias = np.asarray(bias, dtype=np.float32)
    edge_weight = np.asarray(edge_weight, dtype=np.float32)
    edge_index = np.asarray(edge_index)

    pre = _preprocess(edge_index, edge_weight)
    COLS, ncalls = pre["COLS"], pre["ncalls"]

    ck = (COLS, tuple(pre["off"].tolist()))
    if ck not in _CACHE:
        _CACHE[ck] = _build_nc(COLS, pre["off"], pre["call_plan"])
    nc = _CACHE[ck]

    node_at_rank = pre["node_at_rank"]
    in_maps = []
    for c in range(N_CORES):
        xp = np.zeros((PADN, D), dtype=np.float32)
        real = node_at_rank[c] >= 0
        xp[real] = x[node_at_rank[c][real]]
        in_maps.append({
            "x": xp,
            "gidx": np.ascontiguousarray(pre["gidx_w"][c]),
            "s2": np.ascontiguousarray(pre["s2"][c])[..., None].astype(ml_dtypes.bfloat16),
            "W": W,
            "bias": bias.reshape(1, D),
        })

    trace = bool(int(os.environ.get("BASS_GNN_TRACE", "0")))
    res = run_bass_kernel_spmd(nc, in_maps, core_ids=list(range(N_CORES)),
                               trace=trace)
    if trace:
        kernel.last_exec_ns = res.exec_time_ns
        kernel.last_trace = (res.instructions_and_trace[1]
                             if res.instructions_and_trace else None)

    out = np.zeros((N_NODES, D), dtype=np.float32)
    for c in range(N_CORES):
        oc = res.results[c]["out"]
        real = node_at_rank[c] >= 0
        out[node_at_rank[c][real]] = oc[real]
    return out


# revision 25
# speedup vs baseline: 1.0624x; 1.0624x over previous
"""AdaptiveEdgeWeightGNN (GCNConv with edge weights) on 8 Trainium2 NeuronCores.

Strategy: destination-sharded edge-parallel with an ELLPACK slot grid.
Nodes are ranked by merged in-degree globally; rank r -> core r%8, window
(r//8)//128, partition (r//8)%128 — so all cores share one tight window
profile (one SPMD program). Host does pure index/layout work (self-loops,
parallel-edge merge, slot assignment, int16 pair indices into a [25088, 128]
paired view of the bf16 y-table). Device: degree reduce -> dis = deg^-1/2 ->
y = dis*x (own slice, bf16) -> AllGather -> staged dma_gather (4 SWDGE
queues) -> DVE pair-select scale -> TensorE identity-stationary PSUM
accumulation -> transpose -> @W -> dis-scale + bias.
"""
import os
import ml_dtypes
import numpy as np

import concourse.bacc as bacc
import concourse.bass as bass
import concourse.tile as tile
from concourse import mybir
from concourse.bass_utils import run_bass_kernel_spmd
from concourse.masks import make_identity

N_NODES = 50000
N_EDGES = 800000
D = 64
N_CORES = 8
NPC = 6250            # real nodes per core
PADN = 6272           # padded nodes per core (49 windows x 128)
N_WIN = PADN // 128   # 49
P = 128
CC = 16               # gather-call width in slot-columns

F32 = mybir.dt.float32
BF16 = mybir.dt.bfloat16
I16 = mybir.dt.int16


def _preprocess(edge_index: np.ndarray, edge_weight: np.ndarray):
    """Pure index/layout preprocessing. Returns per-core grids + permutation."""
    row = np.asarray(edge_index[0], dtype=np.int64)
    col = np.asarray(edge_index[1], dtype=np.int64)
    ew = np.asarray(edge_weight, dtype=np.float64)

    # self-loops (weight 1.0) — matches gcn_norm's add_self_loops
    loop = np.arange(N_NODES, dtype=np.int64)
    row = np.concatenate([row, loop])
    col = np.concatenate([col, loop])
    ew = np.concatenate([ew, np.ones(N_NODES)])

    # merge parallel edges by (dst, src)
    key = col * N_NODES + row
    order0 = np.argsort(key, kind="stable")
    ks = key[order0]
    uniq_mask = np.empty(ks.shape, dtype=bool)
    uniq_mask[0] = True
    uniq_mask[1:] = ks[1:] != ks[:-1]
    seg_id = np.cumsum(uniq_mask) - 1
    ew_m = np.bincount(seg_id, weights=ew[order0])
    ku = ks[uniq_mask]
    dst_m = ku // N_NODES
    src_m = ku % N_NODES

    # merged in-degree (slot count) per node; global degree ranking
    cnt = np.bincount(dst_m, minlength=N_NODES)
    grank_order = np.argsort(-cnt, kind="stable")   # rank -> node
    grank = np.empty(N_NODES, dtype=np.int64)       # node -> rank
    grank[grank_order] = np.arange(N_NODES)

    owner = grank % N_CORES                         # node -> core
    lrank = grank // N_CORES                        # node -> local rank (< 6250)
    perm = owner * PADN + lrank                     # node -> y-table row

    # common per-window K: global sorted counts, max of each 1024-stripe
    csort = cnt[grank_order]                        # descending
    K = np.zeros(N_WIN, dtype=np.int64)
    for w in range(N_WIN):
        s = w * 128 * N_CORES
        K[w] = csort[s] if s < N_NODES else 1
    K = np.maximum(K, 1)
    off = np.zeros(N_WIN + 1, dtype=np.int64)
    off[1:] = np.cumsum(K)
    cols_raw = int(off[-1])
    COLS = ((cols_raw + CC - 1) // CC) * CC

    # slot fill
    gidx = np.zeros((N_CORES, P, COLS), dtype=np.int16)
    slo = np.zeros((N_CORES, P, COLS), dtype=np.float32)
    shi = np.zeros((N_CORES, P, COLS), dtype=np.float32)

    own_m = owner[dst_m]
    lr_m = lrank[dst_m]
    wn = lr_m // 128
    pp = lr_m - wn * 128
    dst_seg_start = np.searchsorted(dst_m, dst_m)   # dst_m sorted
    j = np.arange(dst_m.size) - dst_seg_start
    colpos = off[wn] + j
    assert (j < K[wn]).all()

    prow = perm[src_m]
    pidx = (prow >> 1).astype(np.int16)
    par = (prow & 1).astype(bool)
    gidx[own_m, pp, colpos] = pidx
    slo[own_m, pp, colpos] = np.where(~par, ew_m, 0.0).astype(np.float32)
    shi[own_m, pp, colpos] = np.where(par, ew_m, 0.0).astype(np.float32)
    s2 = np.stack([slo, shi], axis=-1)          # [8, P, COLS, 2]

    # call plan: CC-wide calls, but split the last 16 columns into 8
    # two-column calls so the end-of-stream drain trickle is short
    call_plan = []
    c = 0
    while c < COLS:
        rem = COLS - c
        cc = CC if rem > 16 else 2
        call_plan.append((c, cc))
        c += cc
    ncalls = len(call_plan)
    # wrapped int16 index layout per gather call, concatenated
    parts = []
    for (c0, cc) in call_plan:
        blk = gidx[:, :, c0:c0 + cc]                       # [8, 128, cc]
        flat = blk.transpose(0, 2, 1).reshape(N_CORES, cc * 128)
        w16 = flat.reshape(N_CORES, cc * 8, 16).transpose(0, 2, 1)  # [8,16,cc*8]
        parts.append(np.tile(w16, (1, 8, 1)))
    gidx_w = np.concatenate(parts, axis=2)

    # rank -> node map per core for upload/unshard
    node_at_rank = np.full((N_CORES, PADN), -1, dtype=np.int64)
    for c in range(N_CORES):
        node_at_rank[c, :NPC] = grank_order[c::N_CORES]

    return dict(
        COLS=COLS, K=K, off=off, ncalls=ncalls, call_plan=call_plan,
        gidx_w=gidx_w, s2=s2,
        node_at_rank=node_at_rank,
    )


def _build_nc(COLS: int, off: np.ndarray, call_plan):
    nc = bacc.Bacc("TRN2", target_bir_lowering=False, debug=False,
                   num_devices=N_CORES, num_swdge_queues=4)
    x_in = nc.dram_tensor("x", [PADN, D], F32, kind="ExternalInput")
    gi_in = nc.dram_tensor("gidx", [P, COLS * 8], I16, kind="ExternalInput")
    s2_in = nc.dram_tensor("s2", [P, COLS, 2, 1], BF16, kind="ExternalInput")
    w_in = nc.dram_tensor("W", [D, D], F32, kind="ExternalInput")
    b_in = nc.dram_tensor("bias", [1, D], F32, kind="ExternalInput")
    out_t = nc.dram_tensor("out", [PADN, D], F32, kind="ExternalOutput")

    with tile.TileContext(nc) as tc:
        with tc.tile_pool(name="const", bufs=1) as cp, \
             tc.tile_pool(name="work", bufs=1) as wp, \
             tc.tile_pool(name="work2", bufs=2) as wp2, \
             tc.tile_pool(name="gq", bufs=8) as gq, \
             tc.tile_pool(name="mq", bufs=8) as mq, \
             tc.tile_pool(name="pa", bufs=1, space="PSUM") as pa, \
             tc.tile_pool(name="ps", bufs=1, space="PSUM") as ps, \
             tc.tile_pool(name="dram", bufs=1, space="DRAM") as dp:

            # ---- constants / small uploads ----
            ident_f = cp.tile([P, P], F32, tag="idf")
            make_identity(nc, ident_f[:])
            ident_b = cp.tile([P, P], BF16, tag="idb")
            nc.vector.tensor_copy(ident_b[:], ident_f[:])
            w_sb = cp.tile([D, D], F32, tag="w")
            nc.sync.dma_start(w_sb[:], w_in[:])
            b_row = cp.tile([1, D], F32, tag="brow")
            nc.sync.dma_start(b_row[:], b_in[:])
            ones1 = cp.tile([1, P], F32, tag="ones")
            nc.vector.memset(ones1[:], 1.0)
            bias_ps = ps.tile([P, D], F32, tag="small")
            nc.tensor.matmul(out=bias_ps[:], lhsT=ones1[:], rhs=b_row[:],
                             start=True, stop=True)
            bias_bc = cp.tile([P, 1, D], F32, tag="biasbc")
            nc.vector.tensor_copy(bias_bc[:, 0, :], bias_ps[:])

            s2_t = cp.tile([P, COLS, 2, 1], BF16, tag="s2")
            nc.sync.dma_start(s2_t[:], s2_in[:])
            gi_t = cp.tile([P, COLS * 8], I16, tag="gi")
            nc.sync.dma_start(gi_t[:], gi_in[:])

            # ---- degree / dis ----
            ew_tot = wp.tile([P, COLS, 1], F32, tag="ewt")
            nc.vector.tensor_tensor(out=ew_tot[:], in0=s2_t[:, :, 0, :],
                                    in1=s2_t[:, :, 1, :],
                                    op=mybir.AluOpType.add)
            deg = wp.tile([P, N_WIN, 1], F32, tag="deg")
            for w in range(N_WIN):
                nc.vector.tensor_reduce(
                    out=deg[:, w, :], in_=ew_tot[:, int(off[w]):int(off[w + 1]), :],
                    axis=mybir.AxisListType.XY, op=mybir.AluOpType.add)
            mask = wp.tile([P, N_WIN, 1], F32, tag="mask")
            nc.vector.tensor_scalar(out=mask[:], in0=deg[:], scalar1=0.0,
                                    scalar2=None, op0=mybir.AluOpType.is_gt)
            degp = wp.tile([P, N_WIN, 1], F32, tag="degp")
            nc.vector.tensor_scalar(out=degp[:], in0=deg[:], scalar1=-1.0,
                                    scalar2=None, op0=mybir.AluOpType.add)
            nc.vector.tensor_tensor(out=degp[:], in0=degp[:], in1=mask[:],
                                    op=mybir.AluOpType.mult)
            nc.vector.tensor_scalar(out=degp[:], in0=degp[:], scalar1=1.0,
                                    scalar2=None, op0=mybir.AluOpType.add)
            rec = wp.tile([P, N_WIN, 1], F32, tag="rec")
            nc.vector.reciprocal(rec[:], degp[:])
            dis = wp.tile([P, N_WIN, 1], F32, tag="dis")
            nc.scalar.sqrt(dis[:], rec[:])
            nc.vector.tensor_tensor(out=dis[:], in0=dis[:], in1=mask[:],
                                    op=mybir.AluOpType.mult)

            # ---- y slice build (bf16) + AllGather ----
            x_sb = wp.tile([P, N_WIN, D], F32, tag="xsb")
            nc.sync.dma_start(
                x_sb[:], x_in[:].rearrange("(w p) f -> p w f", p=P))
            y_sb = wp.tile([P, N_WIN, D], BF16, tag="ysb")
            nc.vector.tensor_tensor(out=y_sb[:], in0=x_sb[:],
                                    in1=dis[:].to_broadcast([P, N_WIN, D]),
                                    op=mybir.AluOpType.mult)
            ag_in = dp.tile([PADN, D], BF16, tag="agin")
            nc.gpsimd.dma_start(
                ag_in[:].rearrange("(w p) f -> p w f", p=P), y_sb[:])
            y_full = dp.tile([N_CORES * PADN, D], BF16, tag="yfull")
            nc.gpsimd.collective_compute(
                "AllGather", mybir.AluOpType.bypass,
                replica_groups=[list(range(N_CORES))],
                ins=[ag_in.opt()], outs=[y_full.opt()])
            y_pair = y_full[:].rearrange("(a two) f -> a (two f)", two=2)

            agg = pa.tile([P, N_WIN * D], F32, tag="agg")
            out_raw = wp.tile([P, N_WIN, D], F32, tag="outraw")
            first_col = {int(off[w]): w for w in range(N_WIN)}
            last_col = {int(off[w + 1]) - 1: w for w in range(N_WIN)}
            WPB = 8  # windows per PSUM bank
            n_banks = (N_WIN + WPB - 1) // WPB

            def bank_tail(b):
                """Evict retired bank b, transpose, @W, dis-scale, bias, DMA."""
                w0 = b * WPB
                bw = min(WPB, N_WIN - w0)
                tmp = wp2.tile([P, WPB * D], F32, tag="tmpagg")
                nc.vector.tensor_copy(tmp[:, :bw * D],
                                      agg[:, w0 * D:(w0 + bw) * D])
                for s0 in range(0, bw, 4):
                    sw = min(4, bw - s0)
                    pt = ps.tile([D, 4 * P], F32, tag="small")
                    for i in range(sw):
                        nc.tensor.transpose(
                            out=pt[:, i * P:(i + 1) * P],
                            in_=tmp[:, (s0 + i) * D:(s0 + i + 1) * D],
                            identity=ident_f[:])
                    at = wp2.tile([D, 4 * P], F32, tag="aggT")
                    nc.vector.tensor_copy(at[:, :sw * P], pt[:, :sw * P])
                    for i in range(sw):
                        w = w0 + s0 + i
                        nc.tensor.matmul(out=agg[:, w * D:(w + 1) * D],
                                         lhsT=at[:, i * P:(i + 1) * P],
                                         rhs=w_sb[:], start=True, stop=True)
                nc.vector.tensor_copy(
                    out_raw[:, w0:w0 + bw, :].rearrange("p w f -> p (w f)"),
                    agg[:, w0 * D:(w0 + bw) * D])
                nc.vector.tensor_tensor(
                    out=out_raw[:, w0:w0 + bw, :],
                    in0=out_raw[:, w0:w0 + bw, :],
                    in1=dis[:, w0:w0 + bw, :].to_broadcast([P, bw, D]),
                    op=mybir.AluOpType.mult)
                nc.vector.tensor_tensor(
                    out=out_raw[:, w0:w0 + bw, :],
                    in0=out_raw[:, w0:w0 + bw, :],
                    in1=bias_bc[:].to_broadcast([P, bw, D]),
                    op=mybir.AluOpType.add)
                nc.sync.dma_start(
                    out_t[:].rearrange("(w p) f -> p w f", p=P)[:, w0:w0 + bw, :],
                    out_raw[:, w0:w0 + bw, :])

            next_bank = 0
            gi_off = 0
            for k, (c0, cc) in enumerate(call_plan):
                g = gq.tile([P, CC, 2 * D], BF16, tag="g")
                nc.gpsimd.dma_gather(
                    out_ap=g[:, :cc, :], in_ap=y_pair,
                    idxs_ap=gi_t[:, gi_off:gi_off + cc * 8],
                    num_idxs=cc * P, num_idxs_reg=cc * P,
                    elem_size=2 * D, single_packet=False, queue_num=k % 4)
                gi_off += cc * 8
                m = mq.tile([P, CC, 2, D], BF16, tag="m")
                nc.vector.tensor_tensor(
                    out=m[:, :cc, :, :],
                    in0=g[:, :cc, :].rearrange("p c (two f) -> p c two f", two=2),
                    in1=s2_t[:, c0:c0 + cc, :, :].to_broadcast([P, cc, 2, D]),
                    op=mybir.AluOpType.mult)
                for cl in range(cc):
                    col = c0 + cl
                    if col >= int(off[-1]):
                        break
                    w = int(np.searchsorted(off, col, side="right")) - 1
                    nc.tensor.matmul(out=agg[:, w * D:(w + 1) * D],
                                     lhsT=ident_b[:], rhs=m[:, cl, 0, :],
                                     start=(col in first_col), stop=False)
                    nc.tensor.matmul(out=agg[:, w * D:(w + 1) * D],
                                     lhsT=ident_b[:], rhs=m[:, cl, 1, :],
                                     start=False, stop=(col in last_col))
                # emit tails for banks fully retired by this call
                while next_bank < n_banks:
                    wlast = min((next_bank + 1) * WPB, N_WIN) - 1
                    if int(off[wlast + 1]) <= min(c0 + cc, int(off[-1])):
                        bank_tail(next_bank)
                        next_bank += 1
                    else:
                        break
            while next_bank < n_banks:
                bank_tail(next_bank)
                next_bank += 1

    nc.compile()
    return nc


_CACHE: dict = {}


def kernel(x, W, bias, edge_weight, edge_index) -> np.ndarray:
    x = np.asarray(x, dtype=np.float32)
    W = np.asarray(W, dtype=np.float32)
    bias = np.asarray(bias, dtype=np.float32)
    edge_weight = np.asarray(edge_weight, dtype=np.float32)
    edge_index = np.asarray(edge_index)

    pre = _preprocess(edge_index, edge_weight)
    COLS, ncalls = pre["COLS"], pre["ncalls"]

    ck = (COLS, tuple(pre["off"].tolist()))
    if ck not in _CACHE:
        _CACHE[ck] = _build_nc(COLS, pre["off"], pre["call_plan"])
    nc = _CACHE[ck]

    node_at_rank = pre["node_at_rank"]
    in_maps = []
    for c in range(N_CORES):
        xp = np.zeros((PADN, D), dtype=np.float32)
        real = node_at_rank[c] >= 0
        xp[real] = x[node_at_rank[c][real]]
        in_maps.append({
            "x": xp,
            "gidx": np.ascontiguousarray(pre["gidx_w"][c]),
            "s2": np.ascontiguousarray(pre["s2"][c])[..., None].astype(ml_dtypes.bfloat16),
            "W": W,
            "bias": bias.reshape(1, D),
        })

    trace = bool(int(os.environ.get("BASS_GNN_TRACE", "0")))
    res = run_bass_kernel_spmd(nc, in_maps, core_ids=list(range(N_CORES)),
                               trace=trace)
    if trace:
        kernel.last_exec_ns = res.exec_time_ns
        kernel.last_trace = (res.instructions_and_trace[1]
                             if res.instructions_and_trace else None)

    out = np.zeros((N_NODES, D), dtype=np.float32)
    for c in range(N_CORES):
        oc = res.results[c]["out"]
        real = node_at_rank[c] >= 0
        out[node_at_rank[c][real]] = oc[real]
    return out
